# revision 5
# baseline (speedup 1.0000x reference)
"""Haar DWT + 1x1 conv fusion kernel for Trainium2, 8-core data parallel.

The whole nn.Module (2D Haar DWT -> channel-stack -> 1x1 conv 12->64) is
algebraically a single 2x2 stride-2 conv with 3 input / 64 output channels:

  out[b,o,i,j] = sum_{c,pr,pc} W2[o,c,pr,pc] * x[b,c,2i+pr,2j+pc]
  W2[o,c,pr,pc] = 0.5*(w[o,4c] + w[o,4c+1]*sLH + w[o,4c+2]*sHL + w[o,4c+3]*sHH)
    sLH = +1 if pr==0 else -1 ; sHL = +1 if pc==0 else -1 ; sHH = +1 if pr==pc else -1

On device this is a K=12 matmul per pc-parity: the stride-2 moving AP reads raw
interleaved x rows directly (no deinterleave pass). A block-diagonal [12,128]
stationary makes one matmul fill all 128 PSUM partitions = 2 independent groups
of output rows. Batch (32) is sharded 4-per-core across 8 NeuronCores.

Per-core loop: 4 samples x 8 strips of 32 output rows.
  in_strip [12, 8192]: partition p = g*6+c*2+pr, free u*512+j' holds
    x[s, c, 64*st + 32g + 2u + pr, j'], u in [0,16)
  8 psum tiles [128,512] (t=0..7): rows i0+16g+2t+r, r in {0,1}
    matmul pc: out[64g+o, r*256+j] += W2[o,c,pr,pc] * in[g6+c2+pr, (2t+r)*512 + 2j+pc]
  strip [128, 4096]: partition 64g+o, free u*256+j = out[o, i0+16g+u, j]
"""

import numpy as np

import concourse.bacc as bacc
import concourse.tile as tile
import concourse.mybir as mybir
from concourse.bass_utils import run_bass_kernel_spmd

F32 = mybir.dt.float32

N_CORES = 8
B, C, H, W = 32, 3, 512, 512
BS = B // N_CORES           # 4 samples per core
OC = 64                     # output channels
HO, WO = H // 2, W // 2     # 256, 256
RPS = 32                    # output rows per strip (16 per psum group)
N_STRIPS = HO // RPS        # 8 strips per sample
TPS = RPS // 4              # 8 psum tiles per strip

_CACHE = {}


def _build():
    if "nc" in _CACHE:
        return _CACHE["nc"]
    nc = bacc.Bacc("TRN2", target_bir_lowering=False, debug=False,
                   num_devices=N_CORES)
    x_d = nc.dram_tensor("x", [BS, C, H, W], F32, kind="ExternalInput")
    wa_d = nc.dram_tensor("wa", [12, 128], F32, kind="ExternalInput")  # pc=0
    wb_d = nc.dram_tensor("wb", [12, 128], F32, kind="ExternalInput")  # pc=1
    o_d = nc.dram_tensor("out", [BS, OC, HO, WO], F32, kind="ExternalOutput")

    with tile.TileContext(nc) as tc:
        with (
            tc.tile_pool(name="wpool", bufs=1) as wpool,
            tc.tile_pool(name="inpool", bufs=3) as inpool,
            tc.tile_pool(name="psum", bufs=8, space="PSUM") as psum_pool,
            tc.tile_pool(name="outpool", bufs=2) as outpool,
        ):
            wa = wpool.tile([12, 128], F32, tag="wa")
            wb = wpool.tile([12, 128], F32, tag="wb")
            nc.sync.dma_start(out=wa[:], in_=wa_d.ap())
            nc.sync.dma_start(out=wb[:], in_=wb_d.ap())

            for s in range(BS):
                for st in range(N_STRIPS):
                    h0 = 2 * RPS * st  # first x row of the strip
                    in_strip = inpool.tile([12, 512 * RPS // 2], F32, tag="in")
                    for c in range(C):
                        for g in range(2):
                            src = (
                                x_d.ap()[s, c, h0 + RPS * g : h0 + RPS * (g + 1), :]
                                .rearrange("(u pr) j -> pr u j", pr=2)
                            )
                            b = g * 6 + c * 2
                            nc.sync.dma_start(out=in_strip[b : b + 2, :], in_=src)

                    strip = outpool.tile([128, 512 * TPS], F32, tag="strip")
                    mv = in_strip[:].rearrange("p (u j) -> p u j", u=RPS // 2)
                    for t in range(TPS):
                        acc = psum_pool.tile([128, 512], F32, tag="acc")
                        m = mv[:, 2 * t : 2 * t + 2, :]
                        nc.tensor.matmul(
                            acc[:], wa[:], m[:, :, 0::2], start=True, stop=False
                        )
                        nc.tensor.matmul(
                            acc[:], wb[:], m[:, :, 1::2], start=False, stop=True
                        )
                        dst = strip[:, 512 * t : 512 * (t + 1)]
                        if t % 4 == 3:
                            nc.scalar.copy(dst, acc[:])
                        else:
                            nc.vector.tensor_copy(dst, acc[:])

                    odst = (
                        o_d.ap()[s, :, RPS * st : RPS * (st + 1), :]
                        .rearrange("o (g u) j -> g o u j", g=2)
                    )
                    nc.scalar.dma_start(out=odst, in_=strip[:])

    nc.compile()
    _CACHE["nc"] = nc
    return nc


def _fold_weights(w_fusion: np.ndarray):
    """[64,12,1,1] conv weight -> two block-diag lhsT [12,128] (pc=0 / pc=1)."""
    wf = w_fusion[:, :, 0, 0].astype(np.float32)  # [64, 12]
    lhs = [np.zeros((12, 128), np.float32) for _ in range(2)]
    for pc in range(2):
        for c in range(C):
            for pr in range(2):
                sLH = 1.0 if pr == 0 else -1.0
                sHL = 1.0 if pc == 0 else -1.0
                sHH = 1.0 if pr == pc else -1.0
                w2 = 0.5 * (wf[:, 4 * c] + wf[:, 4 * c + 1] * sLH
                            + wf[:, 4 * c + 2] * sHL + wf[:, 4 * c + 3] * sHH)
                for g in range(2):
                    lhs[pc][g * 6 + c * 2 + pr, 64 * g : 64 * g + 64] = w2
    return lhs


def kernel(x: np.ndarray, w_fusion: np.ndarray) -> np.ndarray:
    nc = _build()
    wa, wb = _fold_weights(np.asarray(w_fusion))
    x = np.ascontiguousarray(np.asarray(x), dtype=np.float32)
    in_maps = [
        {"x": x[i * BS : (i + 1) * BS], "wa": wa, "wb": wb}
        for i in range(N_CORES)
    ]
    res = run_bass_kernel_spmd(nc, in_maps, list(range(N_CORES)))
    out = np.concatenate([res.results[i]["out"] for i in range(N_CORES)], axis=0)
    return out
